# revision 1
# baseline (speedup 1.0000x reference)
"""Trainium2 Bass kernel for nn_DA3CrossFrameCFDistanceLoss.

Strategy (8 NeuronCores):
  Phase 1 (data-parallel over batch x extra-frame shard):
    core c -> (b = c//4, shard s = c%4).  Each core streams one teacher
    extra frame transposed (extT [D, 4096] = teacher[b, EXTRA_FRAMES[s]].T),
    computes cosine-similarity sim[r, e] = (ref_n . x_e) * rinv[e] with the
    TensorEngine (float32r matmuls, fp32 PSUM accumulation), and extracts the
    per-shard top-8 values + indices per ref row with the DVE max/max_index
    instructions.  Host merges the 4 shards' top-8 per row into the global
    top-4 (the cross-shard topk merge) and gathers the selected rows.
  Phase 2 (data-parallel over (batch, row-half, feature-half)):
    core c -> (b, h, dh).  Each core computes, for its 128 ref rows and 512
    feature columns, the 19 KL "units":
        d1 (j=0..2):  xt = ref_t - shared_t[j],   xs = ref_s - shared_s[j]
        d2 (k=0..3):  xt = ref_t - sim_high[k],   xs = ref_s - sim_high[k]
        d3 (j,k):     xt = shared_t[j] - simh[k], xs = shared_s[j] - simh[k]
    For each unit it produces partial (Zt, Zs, num) where
        Zt = sum exp(xt), Zs = sum exp(xs), num = sum exp(xt) * (xt - xs)
    using fused ACT exp+accum and DVE tensor_tensor_reduce.  Host combines
    the two feature-halves, evaluates kl = num/Zt - log Zt + log Zs, applies
    SmoothL1 and the weighted averaging.
"""

import numpy as np

import concourse.bass as bass
from concourse import bacc
import concourse.mybir as mybir
from concourse import bass_utils
from concourse.tile import TileContext

# ---- problem constants (hardcoded from the nn.Module defaults) ----
B, V, P, D = 2, 8, 4096, 1024
EXTRA_FRAMES = [1, 3, 5, 7]
SHARED_TEACHER = [2, 4, 6]
SHARED_STUDENT = [1, 2, 3]
NUM_REF = 256
NUM_SHARED = 256
TOPK = 4
TEMP = 1.0
BETA = 0.5
N_CORES = 8

ES = P          # extra rows per shard (one frame per shard)
EB = 1024       # phase-1 e-block size
NBLK = ES // EB
DH = D // 2     # phase-2 feature half
N_UNITS = 19    # 3 d1 + 4 d2 + 12 d3

F32 = mybir.dt.float32
F32R = mybir.dt.float32r
U32 = mybir.dt.uint32

_CACHE = {}

# Results of the most recent launches (exec_time_ns etc), for test harnesses.
LAST_PERF = {}


def _build_phase1():
    nc = bacc.Bacc("TRN2", target_bir_lowering=False, debug=False,
                   enable_asserts=False, num_devices=N_CORES)
    extT = nc.dram_tensor("extT", (D, ES), F32R, kind="ExternalInput").ap()
    refT = nc.dram_tensor("refT", (D, NUM_REF), F32R, kind="ExternalInput").ap()
    rinv = nc.dram_tensor("rinv", (1, ES), F32, kind="ExternalInput").ap()
    vals_o = nc.dram_tensor("vals", (2, 128, 8), F32, kind="ExternalOutput").ap()
    idx_o = nc.dram_tensor("idx", (2, 128, 8), U32, kind="ExternalOutput").ap()

    extT_r = extT.rearrange("(k p) e -> p k e", p=128)
    refT_r = refT.rearrange("(k p) r -> p k r", p=128)

    with TileContext(nc) as tc:
        with (
            tc.tile_pool(name="const", bufs=1) as const_pool,
            tc.tile_pool(name="xin", bufs=3) as xin_pool,
            tc.tile_pool(name="ps", bufs=3, space="PSUM") as ps_pool,
            tc.tile_pool(name="small", bufs=1) as small_pool,
        ):
            refT_sb = const_pool.tile([128, 8, NUM_REF], F32R)
            nc.sync.dma_start(out=refT_sb, in_=refT_r)
            rinv_rep = const_pool.tile([128, ES], F32)
            nc.sync.dma_start(out=rinv_rep, in_=rinv.to_broadcast((128, ES)))
            sim_sb = const_pool.tile([128, 2, ES], F32)
            bv = small_pool.tile([128, 2, NBLK, 8], F32)
            fv = small_pool.tile([128, 2, 8], F32)
            fidx = small_pool.tile([128, 2, 8], U32)

            for eb in range(NBLK):
                esl = slice(eb * EB, (eb + 1) * EB)
                xt = xin_pool.tile([128, 8, EB], F32R, tag="xt")
                nc.sync.dma_start(out=xt, in_=extT_r[:, :, esl])
                # A PE matmul may carry at most one semaphore wait (walrus
                # S3_LW limit).  Consume the xt-DMA dependency with a throwaway
                # matmul so the real matmuls only ever wait on one source.
                dum = ps_pool.tile([128, 512], F32, tag="dum", name="dum", bufs=1)
                nc.tensor.matmul(dum, lhsT=xt[:, 0, 0:128], rhs=xt[:, 0, 0:512],
                                 start=True, stop=True, skip_group_check=True)
                for m in range(2):
                    ps = ps_pool.tile([128, EB], F32, tag="ps", name="ps")
                    for k in range(8):
                        for nn in range(EB // 512):
                            nc.tensor.matmul(
                                ps[:, nn * 512:(nn + 1) * 512],
                                lhsT=refT_sb[:, k, m * 128:(m + 1) * 128],
                                rhs=xt[:, k, nn * 512:(nn + 1) * 512],
                                start=(k == 0), stop=(k == 7),
                            )
                    nc.vector.tensor_mul(sim_sb[:, m, esl], ps, rinv_rep[:, esl])
                    nc.vector.max(out=bv[:, m, eb, :], in_=sim_sb[:, m, esl])
            for m in range(2):
                nc.vector.max(out=fv[:, m, :], in_=bv[:, m, :, :])
                nc.vector.max_index(out=fidx[:, m, :], in_max=fv[:, m, :],
                                    in_values=sim_sb[:, m, :])
                nc.sync.dma_start(out=vals_o[m], in_=fv[:, m, :])
                nc.sync.dma_start(out=idx_o[m], in_=fidx[:, m, :])
    nc.compile()
    return nc


def _phase2_units():
    units = [("d1", j, None) for j in range(3)]
    units += [("d2", None, k) for k in range(4)]
    units += [("d3", j, k) for j in range(3) for k in range(4)]
    return units


def _build_phase2():
    nc = bacc.Bacc("TRN2", target_bir_lowering=False, debug=False,
                   enable_asserts=False, num_devices=N_CORES)
    REF = nc.dram_tensor("ref2", (2, 128, DH), F32, kind="ExternalInput").ap()
    SHT = nc.dram_tensor("sht", (3, 128, DH), F32, kind="ExternalInput").ap()
    SHS = nc.dram_tensor("shs", (3, 128, DH), F32, kind="ExternalInput").ap()
    SIMH = nc.dram_tensor("simh", (4, 128, DH), F32, kind="ExternalInput").ap()
    ZB_o = nc.dram_tensor("zb", (128, N_UNITS, 3), F32, kind="ExternalOutput").ap()

    Exp = mybir.ActivationFunctionType.Exp
    mult = mybir.AluOpType.mult
    add = mybir.AluOpType.add

    with TileContext(nc) as tc:
        with (
            tc.tile_pool(name="src", bufs=1) as src_pool,
            tc.tile_pool(name="work", bufs=3) as work_pool,
        ):
            ref_sb = src_pool.tile([128, 2, DH], F32)
            nc.sync.dma_start(out=ref_sb, in_=REF.rearrange("a p d -> p a d"))
            sht_sb = src_pool.tile([128, 3, DH], F32)
            nc.sync.dma_start(out=sht_sb, in_=SHT.rearrange("a p d -> p a d"))
            shs_sb = src_pool.tile([128, 3, DH], F32)
            nc.sync.dma_start(out=shs_sb, in_=SHS.rearrange("a p d -> p a d"))
            simh_sb = src_pool.tile([128, 4, DH], F32)
            nc.sync.dma_start(out=simh_sb, in_=SIMH.rearrange("a p d -> p a d"))

            zb = src_pool.tile([128, N_UNITS, 3], F32)

            # shared difference tensors: d = xt - xs per unit type
            rd = src_pool.tile([128, DH], F32)
            nc.vector.tensor_sub(rd, ref_sb[:, 0, :], ref_sb[:, 1, :])
            sd = src_pool.tile([128, 3, DH], F32)
            dd1 = src_pool.tile([128, 3, DH], F32)
            for j in range(3):
                nc.gpsimd.tensor_sub(sd[:, j, :], sht_sb[:, j, :], shs_sb[:, j, :])
                nc.vector.tensor_sub(dd1[:, j, :], rd, sd[:, j, :])

            for u, (typ, j, k) in enumerate(_phase2_units()):
                if typ == "d1":
                    At, Bt = ref_sb[:, 0, :], sht_sb[:, j, :]
                    As, Bs = ref_sb[:, 1, :], shs_sb[:, j, :]
                    dap = dd1[:, j, :]
                elif typ == "d2":
                    At, Bt = ref_sb[:, 0, :], simh_sb[:, k, :]
                    As, Bs = ref_sb[:, 1, :], simh_sb[:, k, :]
                    dap = rd
                else:
                    At, Bt = sht_sb[:, j, :], simh_sb[:, k, :]
                    As, Bs = shs_sb[:, j, :], simh_sb[:, k, :]
                    dap = sd[:, j, :]

                xt = work_pool.tile([128, DH], F32, tag="xt", name="xt")
                nc.gpsimd.tensor_sub(xt, At, Bt)
                xs = work_pool.tile([128, DH], F32, tag="xs", name="xs")
                nc.vector.tensor_sub(xs, As, Bs)
                et = work_pool.tile([128, DH], F32, tag="et", name="et")
                nc.scalar.activation(et, xt, Exp, accum_out=zb[:, u, 0:1])
                es = work_pool.tile([128, DH], F32, tag="es", name="es")
                nc.scalar.activation(es, xs, Exp, accum_out=zb[:, u, 1:2])
                w = work_pool.tile([128, DH], F32, tag="w", name="w")
                nc.vector.scalar_tensor_tensor(
                    out=w, in0=et, scalar=1.0, in1=dap,
                    op0=mult, op1=mult, accum_out=zb[:, u, 2:3],
                )

            nc.sync.dma_start(out=ZB_o, in_=zb)
    nc.compile()
    return nc


def _get(name):
    if name not in _CACHE:
        _CACHE[name] = _build_phase1() if name == "p1" else _build_phase2()
    return _CACHE[name]


def kernel(**inputs):
    tf = np.ascontiguousarray(np.asarray(inputs["teacher_feats"], dtype=np.float32))
    sf = np.ascontiguousarray(np.asarray(inputs["student_feats"], dtype=np.float32))
    in_dtype = np.asarray(inputs["ref_perm"]).dtype
    ref_perm = np.asarray(inputs["ref_perm"]).astype(np.int64)[:NUM_REF]
    shared_perm = np.asarray(inputs["shared_perm"]).astype(np.int64)[:NUM_SHARED]
    assert in_dtype == np.int32

    # ---- host gathers + ref normalization (tiny) ----
    ref_t = tf[:, 0, ref_perm, :]                       # [B, 256, 1024]
    ref_s = sf[:, 0, ref_perm, :]
    rn = np.sqrt(np.einsum("brd,brd->br", ref_t, ref_t))[..., None]
    refn = ref_t / np.maximum(rn, 1e-12)
    refTs = [np.ascontiguousarray(refn[b].T) for b in range(B)]

    # ---- phase 1: sharded cosine-sim + per-shard top-8 ----
    in_maps1 = []
    for c in range(N_CORES):
        b, s = divmod(c, 4)
        x = tf[b, EXTRA_FRAMES[s]]                      # [4096, 1024]
        extT = np.ascontiguousarray(x.T)                # [1024, 4096]
        nrm = np.sqrt(np.einsum("ed,ed->e", x, x))
        rinv = (1.0 / np.maximum(nrm, 1e-12)).astype(np.float32)[None, :]
        in_maps1.append({"extT": extT, "refT": refTs[b], "rinv": rinv})

    res1 = bass_utils.run_bass_kernel_spmd(
        _get("p1"), in_maps1, core_ids=list(range(N_CORES)))
    LAST_PERF["p1"] = res1

    # ---- host cross-shard top-k merge ----
    gidx = np.zeros((B, NUM_REF, TOPK), dtype=np.int64)
    for b in range(B):
        vals = np.concatenate(
            [res1.results[b * 4 + s]["vals"].reshape(NUM_REF, 8)
             for s in range(4)], axis=1)                # [256, 32]
        idxs = np.concatenate(
            [res1.results[b * 4 + s]["idx"].reshape(NUM_REF, 8).astype(np.int64)
             + s * ES for s in range(4)], axis=1)
        order = np.argsort(-vals, axis=1, kind="stable")[:, :TOPK]
        gidx[b] = np.take_along_axis(idxs, order, axis=1)

    fr = np.asarray(EXTRA_FRAMES, dtype=np.int64)[gidx // P]
    pt = gidx % P
    sim_high = tf[np.arange(B)[:, None, None], fr, pt]  # [B, 256, 4, 1024]

    # ---- phase 2: distances ----
    sh_t = np.stack([tf[:, t, shared_perm, :] for t in SHARED_TEACHER], axis=1)
    sh_s = np.stack([sf[:, s, shared_perm, :] for s in SHARED_STUDENT], axis=1)

    in_maps2 = []
    for c in range(N_CORES):
        b, h, dh = c >> 2, (c >> 1) & 1, c & 1
        rs = slice(h * 128, (h + 1) * 128)
        cs = slice(dh * DH, (dh + 1) * DH)
        ref2 = np.ascontiguousarray(
            np.stack([ref_t[b, rs, cs], ref_s[b, rs, cs]]))
        sht = np.ascontiguousarray(sh_t[b, :, rs, cs])
        shs = np.ascontiguousarray(sh_s[b, :, rs, cs])
        simh = np.ascontiguousarray(sim_high[b, rs, :, cs].transpose(1, 0, 2))
        in_maps2.append({"ref2": ref2, "sht": sht, "shs": shs, "simh": simh})

    res2 = bass_utils.run_bass_kernel_spmd(
        _get("p2"), in_maps2, core_ids=list(range(N_CORES)))
    LAST_PERF["p2"] = res2

    # ---- host tail: kl + SmoothL1 + averaging ----
    s1 = s2 = s3 = 0.0
    for b in range(B):
        for h in range(2):
            z = (res2.results[b * 4 + h * 2 + 0]["zb"].astype(np.float64)
                 + res2.results[b * 4 + h * 2 + 1]["zb"].astype(np.float64))
            Zt, Zs, num = z[..., 0], z[..., 1], z[..., 2]   # [128, 19]
            kl = num / Zt - np.log(Zt) + np.log(Zs)
            akl = np.abs(kl)
            hub = np.where(akl < BETA, 0.5 * kl * kl / BETA, akl - 0.5 * BETA)
            s1 += hub[:, 0:3].sum()
            s2 += hub[:, 3:7].sum()
            s3 += hub[:, 7:19].sum()

    loss = (s1 / (3 * B * NUM_REF)
            + s2 / (B * NUM_REF * TOPK)
            + s3 / (3 * B * NUM_REF * TOPK))
    return np.float32(loss)



# revision 3
# speedup vs baseline: 1.5990x; 1.5990x over previous
"""Trainium2 Bass kernel for nn_DA3CrossFrameCFDistanceLoss.

Strategy (8 NeuronCores):
  Phase 1 (data-parallel over batch x extra-frame shard):
    core c -> (b = c//4, shard s = c%4).  Host pre-normalizes both the ref
    rows and the extra-frame rows (folding the 1/||x|| scaling into the
    operands) and ships them as bf16, halving HBM traffic.  Each core
    streams its frame transposed (extT [D, 4096] bf16), runs bf16 matmuls
    against the stationary ref block (PSUM fp32 accumulation), and extracts
    per-512-column-block top-8 values + indices per ref row with the DVE
    max/max_index instructions reading PSUM directly.  Host merges the
    8 blocks x 4 shards top-8s per row into the exact global top-4 and
    gathers the selected rows.
  Phase 2 (data-parallel over (batch, row-half, feature-half)):
    core c -> (b, h, dh).  Layout is transposed on host: d sits on the
    partition axis (4 chunks of 128), rows on the free axis.  The 19 KL
    "units" need Zt = sum exp(xt), Zs = sum exp(xs), num = sum exp(xt)*
    (xt - xs).  Using exp(a-b) = exp(a)*exp(-b), the ACT engine computes
    just 18 base exponentials (3 wide instructions); the DVE forms the
    per-unit products Pt, Ps and W = Pt*(xt-xs) as wide stride-0-broadcast
    tensor ops; and the otherwise-idle PE performs all 57 per-unit
    reductions over d as ones-vector matmuls accumulated across the 4
    partition chunks in PSUM.  Host evaluates kl = num/Zt - log Zt +
    log Zs, SmoothL1 and the weighted averaging.
"""

import numpy as np
import ml_dtypes

import concourse.bass as bass
from concourse import bacc
import concourse.mybir as mybir
from concourse import bass_utils
from concourse.tile import TileContext

# ---- problem constants (hardcoded from the nn.Module defaults) ----
B, V, P, D = 2, 8, 4096, 1024
EXTRA_FRAMES = [1, 3, 5, 7]
SHARED_TEACHER = [2, 4, 6]
SHARED_STUDENT = [1, 2, 3]
NUM_REF = 256
NUM_SHARED = 256
TOPK = 4
TEMP = 1.0
BETA = 0.5
N_CORES = 8

ES = P          # extra rows per shard (one frame per shard)
EB = 512        # phase-1 e-block size (one PSUM bank of fp32)
NBLK = ES // EB
DH = D // 2     # phase-2 feature half
NC4 = DH // 128  # phase-2 d chunks per core
N_UNITS = 19    # 3 d1 + 4 d2 + 12 d3

F32 = mybir.dt.float32
BF16 = mybir.dt.bfloat16
FP16 = mybir.dt.float16
U32 = mybir.dt.uint32

BF16_NP = ml_dtypes.bfloat16

_CACHE = {}

# Results of the most recent launches (exec_time_ns etc), for test harnesses.
LAST_PERF = {}


def _build_phase1():
    nc = bacc.Bacc("TRN2", target_bir_lowering=False, debug=False,
                   enable_asserts=False, num_devices=N_CORES)
    extT = nc.dram_tensor("extT", (D, ES), BF16, kind="ExternalInput").ap()
    refT = nc.dram_tensor("refT", (D, NUM_REF), BF16, kind="ExternalInput").ap()
    vals_o = nc.dram_tensor("vals", (128, 2, NBLK, 8), F32,
                            kind="ExternalOutput").ap()
    idx_o = nc.dram_tensor("idx", (128, 2, NBLK, 8), U32,
                           kind="ExternalOutput").ap()

    extT_r = extT.rearrange("(k p) e -> p k e", p=128)
    refT_r = refT.rearrange("(k p) r -> p k r", p=128)

    with TileContext(nc) as tc:
        with (
            tc.tile_pool(name="const", bufs=1) as const_pool,
            tc.tile_pool(name="xin", bufs=3) as xin_pool,
            tc.tile_pool(name="ps", bufs=4, space="PSUM") as ps_pool,
            tc.tile_pool(name="dum", bufs=1, space="PSUM") as dum_pool,
            tc.tile_pool(name="small", bufs=1) as small_pool,
        ):
            refT_sb = const_pool.tile([128, 8, NUM_REF], BF16)
            nc.sync.dma_start(out=refT_sb, in_=refT_r)
            fv = small_pool.tile([128, 2, NBLK, 8], F32)
            fidx = small_pool.tile([128, 2, NBLK, 8], U32)
            dum = dum_pool.tile([128, 8], F32)

            for eb in range(NBLK):
                esl = slice(eb * EB, (eb + 1) * EB)
                xt = xin_pool.tile([128, 8, EB], BF16, tag="xt")
                nc.sync.dma_start(out=xt, in_=extT_r[:, :, esl])
                # A PE matmul may carry at most one semaphore wait (walrus
                # S3_LW limit).  Consume the xt-DMA dependency with a tiny
                # throwaway matmul so the real matmuls only ever wait on one
                # source (the PSUM tile WAR release).
                nc.tensor.matmul(dum, lhsT=xt[:, 0, 0:128], rhs=xt[:, 0, 0:8],
                                 start=True, stop=True, skip_group_check=True)
                for m in range(2):
                    ps = ps_pool.tile([128, EB], F32, tag="ps", name="ps")
                    for k in range(8):
                        nc.tensor.matmul(
                            ps,
                            lhsT=refT_sb[:, k, m * 128:(m + 1) * 128],
                            rhs=xt[:, k, :],
                            start=(k == 0), stop=(k == 7),
                        )
                    nc.vector.max(out=fv[:, m, eb, :], in_=ps)
                    nc.vector.max_index(out=fidx[:, m, eb, :],
                                        in_max=fv[:, m, eb, :], in_values=ps)
            nc.sync.dma_start(out=vals_o, in_=fv)
            nc.sync.dma_start(out=idx_o, in_=fidx)
    nc.compile()
    return nc


def _phase2_units():
    units = [("d1", j, None) for j in range(3)]
    units += [("d2", None, k) for k in range(4)]
    units += [("d3", j, k) for j in range(3) for k in range(4)]
    return units


def _build_phase2():
    # Transposed layout: partition axis = d (4 chunks of 128), free = rows.
    # Input S slot order: rt=0, sht=1..3, rs=4, shs=5..7, simh=8..11.
    nc = bacc.Bacc("TRN2", target_bir_lowering=False, debug=False,
                   enable_asserts=False, num_devices=N_CORES)
    S_in = nc.dram_tensor("sin", (128, 12, NC4, 128), FP16,
                          kind="ExternalInput").ap()
    ones_in = nc.dram_tensor("ones", (128, 1), FP16, kind="ExternalInput").ap()
    ZB_o = nc.dram_tensor("zb", (128, 3 * N_UNITS), F32,
                          kind="ExternalOutput").ap()

    Exp = mybir.ActivationFunctionType.Exp

    with TileContext(nc) as tc:
        with (
            tc.tile_pool(name="src", bufs=1) as src_pool,
            tc.tile_pool(name="zps", bufs=1, space="PSUM") as zps_pool,
        ):
            S = src_pool.tile([128, 12, NC4, 128], FP16)
            nc.sync.dma_start(out=S, in_=S_in)
            ones = src_pool.tile([128, 1], FP16)
            nc.sync.dma_start(out=ones, in_=ones_in)

            # base exponentials: Epos = exp(S[0:8]); Eneg split in two
            # (slot 4 = rs is never negated).
            Epos = src_pool.tile([128, 8, NC4, 128], FP16)
            nc.scalar.activation(Epos, S[:, 0:8], Exp)
            # Eneg slots: sht- = 0..2, shs- = 3..5, simh- = 6..9
            Eneg = src_pool.tile([128, 10, NC4, 128], FP16)
            nc.scalar.activation(Eneg[:, 0:3], S[:, 1:4], Exp, scale=-1.0)
            nc.scalar.activation(Eneg[:, 3:10], S[:, 5:12], Exp, scale=-1.0)

            # base diffs for dap = xt - xs (on the Pool engine; DVE is busy)
            rd = src_pool.tile([128, 1, NC4, 128], FP16)
            nc.gpsimd.tensor_sub(rd, S[:, 0:1], S[:, 4:5])
            sd = src_pool.tile([128, 3, NC4, 128], FP16)
            nc.gpsimd.tensor_sub(sd, S[:, 1:4], S[:, 5:8])
            dd1 = src_pool.tile([128, 3, NC4, 128], FP16)
            nc.gpsimd.tensor_sub(dd1, rd.broadcast_to((128, 3, NC4, 128)), sd)

            def b4(ap, n):  # broadcast a [128,1,c,r] slice over n units
                return ap.broadcast_to((128, n, NC4, 128))

            # per-unit products; unit order: d1 j0..2, d2 k0..3, d3 (j,k)
            Pt = src_pool.tile([128, N_UNITS, NC4, 128], FP16)
            nc.vector.tensor_mul(Pt[:, 0:3], b4(Epos[:, 0:1], 3), Eneg[:, 0:3])
            nc.vector.tensor_mul(Pt[:, 3:7], b4(Epos[:, 0:1], 4), Eneg[:, 6:10])
            sht_e = Epos[:, 1:4].rearrange("p j (o c) r -> p j o c r", o=1) \
                .broadcast_to((128, 3, 4, NC4, 128))
            simh_e = Eneg[:, 6:10].rearrange("p (o k) c r -> p o k c r", o=1) \
                .broadcast_to((128, 3, 4, NC4, 128))
            Pt_d3 = Pt[:, 7:19].rearrange("p (j k) c r -> p j k c r", j=3)
            nc.vector.tensor_mul(Pt_d3, sht_e, simh_e)

            Ps = src_pool.tile([128, N_UNITS, NC4, 128], FP16)
            nc.vector.tensor_mul(Ps[:, 0:3], b4(Epos[:, 4:5], 3), Eneg[:, 3:6])
            nc.vector.tensor_mul(Ps[:, 3:7], b4(Epos[:, 4:5], 4), Eneg[:, 6:10])
            shs_e = Epos[:, 5:8].rearrange("p j (o c) r -> p j o c r", o=1) \
                .broadcast_to((128, 3, 4, NC4, 128))
            Ps_d3 = Ps[:, 7:19].rearrange("p (j k) c r -> p j k c r", j=3)
            nc.vector.tensor_mul(Ps_d3, shs_e, simh_e)

            W = src_pool.tile([128, N_UNITS, NC4, 128], FP16)
            nc.vector.tensor_mul(W[:, 0:3], Pt[:, 0:3], dd1)
            nc.vector.tensor_mul(W[:, 3:7], Pt[:, 3:7], b4(rd, 4))
            sd_e = sd.rearrange("p j (o c) r -> p j o c r", o=1) \
                .broadcast_to((128, 3, 4, NC4, 128))
            W_d3 = W[:, 7:19].rearrange("p (j k) c r -> p j k c r", j=3)
            nc.vector.tensor_mul(W_d3, Pt_d3, sd_e)

            # All 57 reductions over d on the PE: ones-vector matmuls,
            # accumulated across the NC4 partition chunks in PSUM.
            Z = zps_pool.tile([128, 3 * N_UNITS], F32)
            for t, T in enumerate((Pt, Ps, W)):
                for u in range(N_UNITS):
                    col = t * N_UNITS + u
                    for c in range(NC4):
                        nc.tensor.matmul(
                            Z[:, col:col + 1],
                            lhsT=T[:, u, c, :],
                            rhs=ones,
                            start=(c == 0), stop=(c == NC4 - 1),
                        )
            zsb = src_pool.tile([128, 3 * N_UNITS], F32)
            nc.scalar.copy(zsb, Z)
            nc.sync.dma_start(out=ZB_o, in_=zsb)
    nc.compile()
    return nc


def _get(name):
    if name not in _CACHE:
        _CACHE[name] = _build_phase1() if name == "p1" else _build_phase2()
    return _CACHE[name]


def kernel(**inputs):
    tf = np.ascontiguousarray(np.asarray(inputs["teacher_feats"], dtype=np.float32))
    sf = np.ascontiguousarray(np.asarray(inputs["student_feats"], dtype=np.float32))
    in_dtype = np.asarray(inputs["ref_perm"]).dtype
    ref_perm = np.asarray(inputs["ref_perm"]).astype(np.int64)[:NUM_REF]
    shared_perm = np.asarray(inputs["shared_perm"]).astype(np.int64)[:NUM_SHARED]
    assert in_dtype == np.int32

    # ---- host gathers + normalization (tiny) ----
    ref_t = tf[:, 0, ref_perm, :]                       # [B, 256, 1024]
    ref_s = sf[:, 0, ref_perm, :]
    rn = np.sqrt(np.einsum("brd,brd->br", ref_t, ref_t))[..., None]
    refn = ref_t / np.maximum(rn, 1e-12)
    refTs = [np.ascontiguousarray(refn[b].T.astype(BF16_NP)) for b in range(B)]

    # ---- phase 1: sharded cosine-sim + per-block top-8 ----
    in_maps1 = []
    for c in range(N_CORES):
        b, s = divmod(c, 4)
        x = tf[b, EXTRA_FRAMES[s]]                      # [4096, 1024]
        nrm = np.sqrt(np.einsum("ed,ed->e", x, x))
        xn = x / np.maximum(nrm, 1e-12)[:, None]
        extT = np.ascontiguousarray(xn.T.astype(BF16_NP))   # [1024, 4096]
        in_maps1.append({"extT": extT, "refT": refTs[b]})

    res1 = bass_utils.run_bass_kernel_spmd(
        _get("p1"), in_maps1, core_ids=list(range(N_CORES)))
    LAST_PERF["p1"] = res1

    # ---- host cross-block/cross-shard top-k merge (exact) ----
    gidx = np.zeros((B, NUM_REF, TOPK), dtype=np.int64)
    for b in range(B):
        # vals [128, 2, NBLK, 8] per shard; row r = m*128 + p
        vals = np.concatenate(
            [res1.results[b * 4 + s]["vals"].transpose(1, 0, 2, 3)
             .reshape(NUM_REF, NBLK * 8) for s in range(4)], axis=1)
        idxs = np.concatenate(
            [(res1.results[b * 4 + s]["idx"].astype(np.int64)
              + (np.arange(NBLK) * EB)[None, None, :, None]
              + s * ES).transpose(1, 0, 2, 3).reshape(NUM_REF, NBLK * 8)
             for s in range(4)], axis=1)
        order = np.argsort(-vals, axis=1, kind="stable")[:, :TOPK]
        gidx[b] = np.take_along_axis(idxs, order, axis=1)

    fr = np.asarray(EXTRA_FRAMES, dtype=np.int64)[gidx // P]
    pt = gidx % P
    sim_high = tf[np.arange(B)[:, None, None], fr, pt]  # [B, 256, 4, 1024]

    # ---- phase 2: distances ----
    sh_t = np.stack([tf[:, t, shared_perm, :] for t in SHARED_TEACHER], axis=1)
    sh_s = np.stack([sf[:, s, shared_perm, :] for s in SHARED_STUDENT], axis=1)

    def t_chunks(a):  # [128 rows, DH] -> [128 dpart, NC4, 128 rows]
        return a.T.reshape(NC4, 128, 128).transpose(1, 0, 2)

    ones_arr = np.ones((128, 1), dtype=np.float16)
    in_maps2 = []
    for c in range(N_CORES):
        b, h, dh = c >> 2, (c >> 1) & 1, c & 1
        rs = slice(h * 128, (h + 1) * 128)
        cs = slice(dh * DH, (dh + 1) * DH)
        # S slot order: rt, sht0..2, rs, shs0..2, simh0..3
        slots = ([ref_t[b, rs, cs]]
                 + [sh_t[b, j, rs, cs] for j in range(3)]
                 + [ref_s[b, rs, cs]]
                 + [sh_s[b, j, rs, cs] for j in range(3)]
                 + [sim_high[b, rs, k, cs] for k in range(4)])
        S = np.stack([t_chunks(a) for a in slots], axis=1)  # [128,12,NC4,128]
        in_maps2.append({"sin": np.ascontiguousarray(S.astype(np.float16)),
                         "ones": ones_arr})

    res2 = bass_utils.run_bass_kernel_spmd(
        _get("p2"), in_maps2, core_ids=list(range(N_CORES)))
    LAST_PERF["p2"] = res2

    # ---- host tail: kl + SmoothL1 + averaging ----
    s1 = s2 = s3 = 0.0
    for b in range(B):
        for h in range(2):
            z = (res2.results[b * 4 + h * 2 + 0]["zb"].astype(np.float64)
                 + res2.results[b * 4 + h * 2 + 1]["zb"].astype(np.float64))
            Zt = z[:, 0:N_UNITS]
            Zs = z[:, N_UNITS:2 * N_UNITS]
            num = z[:, 2 * N_UNITS:3 * N_UNITS]          # [128, 19]
            kl = num / Zt - np.log(Zt) + np.log(Zs)
            akl = np.abs(kl)
            hub = np.where(akl < BETA, 0.5 * kl * kl / BETA, akl - 0.5 * BETA)
            s1 += hub[:, 0:3].sum()
            s2 += hub[:, 3:7].sum()
            s3 += hub[:, 7:19].sum()

    loss = (s1 / (3 * B * NUM_REF)
            + s2 / (B * NUM_REF * TOPK)
            + s3 / (3 * B * NUM_REF * TOPK))
    return np.float32(loss)


# revision 5
# speedup vs baseline: 1.8431x; 1.1526x over previous
"""Trainium2 Bass kernel for nn_DA3CrossFrameCFDistanceLoss.

Strategy (8 NeuronCores):
  Phase 1 (data-parallel over batch x extra-frame shard):
    core c -> (b = c//4, shard s = c%4).  Host pre-normalizes both the ref
    rows and the extra-frame rows (folding the 1/||x|| scaling into the
    operands), scales by 64 and ships them as fp8e4m3, quartering HBM
    traffic.  Each core streams its frame transposed (extT [D, 4096] fp8),
    runs DoubleRow fp8 matmuls (256-deep contraction per instruction, 2x PE
    rate) against the stationary ref block with fp32 PSUM accumulation.
    The ACT engine copies each PSUM block to SBUF as bf16 and the DVE
    extracts per-512-column-block top-8 values + indices per ref row
    (max/max_index).  The host merges the 8 blocks x 4 shards top-8s per
    row into the global top-4 (selection is approximate only through the
    fp8/bf16 rounding of the similarity scores; the merge itself is exact)
    and gathers the selected rows at full fp32 precision.
  Phase 2 (data-parallel over (batch, row-half, feature-half)):
    core c -> (b, h, dh).  Layout is transposed on host: d sits on the
    partition axis (4 chunks of 128), rows on the free axis.  The 19 KL
    "units" need Zt = sum exp(xt), Zs = sum exp(xs), num = sum exp(xt)*
    (xt - xs).  Using exp(a-b) = exp(a)*exp(-b), the ACT engine computes
    just 18 base exponentials (2 wide instructions per chunk); the DVE
    forms the per-unit products Pt, Ps and W = Pt*(xt-xs) as wide
    stride-0-broadcast tensor ops; and the otherwise-idle PE performs all
    57 per-unit reductions over d as ones-vector matmuls accumulated
    across the 4 partition chunks in PSUM.  Everything is pipelined per
    d-chunk so DMA/ACT/DVE/Pool/PE overlap.  Host evaluates
    kl = num/Zt - log Zt + log Zs, SmoothL1 and the weighted averaging.
"""

import numpy as np
import ml_dtypes

import concourse.bass as bass
from concourse import bacc
import concourse.mybir as mybir
from concourse import bass_utils
from concourse.tile import TileContext

# ---- problem constants (hardcoded from the nn.Module defaults) ----
B, V, P, D = 2, 8, 4096, 1024
EXTRA_FRAMES = [1, 3, 5, 7]
SHARED_TEACHER = [2, 4, 6]
SHARED_STUDENT = [1, 2, 3]
NUM_REF = 256
NUM_SHARED = 256
TOPK = 4
TEMP = 1.0
BETA = 0.5
N_CORES = 8

ES = P          # extra rows per shard (one frame per shard)
EB = 512        # phase-1 e-block size (one PSUM bank of fp32)
NBLK = ES // EB
DH = D // 2     # phase-2 feature half
NC4 = DH // 128  # phase-2 d chunks per core
N_UNITS = 19    # 3 d1 + 4 d2 + 12 d3

F32 = mybir.dt.float32
BF16 = mybir.dt.bfloat16
FP16 = mybir.dt.float16
FP8 = mybir.dt.float8e4
U32 = mybir.dt.uint32

BF16_NP = ml_dtypes.bfloat16
FP8_NP = mybir.dt.np(FP8)

_CACHE = {}

# Results of the most recent launches (exec_time_ns etc), for test harnesses.
LAST_PERF = {}


def _build_phase1():
    nc = bacc.Bacc("TRN2", target_bir_lowering=False, debug=False,
                   enable_asserts=False, num_devices=N_CORES)
    extT = nc.dram_tensor("extT", (D, ES), FP8, kind="ExternalInput").ap()
    refT = nc.dram_tensor("refT", (D, NUM_REF), FP8, kind="ExternalInput").ap()
    vals_o = nc.dram_tensor("vals", (128, 2, NBLK, 8), BF16,
                            kind="ExternalOutput").ap()
    idx_o = nc.dram_tensor("idx", (128, 2, NBLK, 8), U32,
                           kind="ExternalOutput").ap()

    extT_r = extT.rearrange("(k p) e -> p k e", p=128)
    refT_r = refT.rearrange("(k p) r -> p k r", p=128)
    DR = mybir.MatmulPerfMode.DoubleRow

    with TileContext(nc) as tc:
        with (
            tc.tile_pool(name="const", bufs=1) as const_pool,
            tc.tile_pool(name="xin", bufs=3) as xin_pool,
            tc.tile_pool(name="sb", bufs=4) as sb_pool,
            tc.tile_pool(name="ps", bufs=4, space="PSUM") as ps_pool,
            tc.tile_pool(name="dum", bufs=1, space="PSUM") as dum_pool,
            tc.tile_pool(name="small", bufs=1) as small_pool,
        ):
            refT_sb = const_pool.tile([128, 8, NUM_REF], FP8)
            nc.sync.dma_start(out=refT_sb, in_=refT_r)
            fv = small_pool.tile([128, 2, NBLK, 8], BF16)
            fidx = small_pool.tile([128, 2, NBLK, 8], U32)
            dum = dum_pool.tile([128, 8], F32)

            for eb in range(NBLK):
                esl = slice(eb * EB, (eb + 1) * EB)
                xt = xin_pool.tile([128, 8, EB], FP8, tag="xt")
                nc.sync.dma_start(out=xt, in_=extT_r[:, :, esl])
                # A PE matmul may carry at most one semaphore wait (walrus
                # S3_LW limit).  Consume the xt-DMA dependency with a tiny
                # throwaway matmul so the real matmuls only ever wait on one
                # source (the PSUM tile WAR release).
                nc.tensor.matmul(dum, lhsT=xt[:, 0, 0:128], rhs=xt[:, 0, 0:8],
                                 start=True, stop=True, skip_group_check=True)
                for m in range(2):
                    ps = ps_pool.tile([128, EB], F32, tag="ps", name="ps")
                    for k2 in range(4):
                        nc.tensor.matmul(
                            ps,
                            lhsT=refT_sb[:, 2 * k2:2 * k2 + 2,
                                         m * 128:(m + 1) * 128],
                            rhs=xt[:, 2 * k2:2 * k2 + 2, :],
                            start=(k2 == 0), stop=(k2 == 3),
                            perf_mode=DR,
                        )
                    sim = sb_pool.tile([128, EB], BF16, tag="sim", name="sim")
                    nc.scalar.copy(sim, ps)
                    nc.vector.max(out=fv[:, m, eb, :], in_=sim)
                    nc.vector.max_index(out=fidx[:, m, eb, :],
                                        in_max=fv[:, m, eb, :], in_values=sim)
            nc.sync.dma_start(out=vals_o, in_=fv)
            nc.sync.dma_start(out=idx_o, in_=fidx)
    nc.compile()
    return nc


def _build_phase2():
    # Transposed layout: partition axis = d (4 chunks of 128), free = rows.
    # Input S slot order: rt=0, rs=1, sht=2..4, shs=5..7, simh=8..11.
    # Epos slots (exp of S[0:8]):  rt=0, rs=1, sht=2..4, shs=5..7
    # Eneg slots (exp of -S[2:12]): sht=0..2, shs=3..5, simh=6..9
    nc = bacc.Bacc("TRN2", target_bir_lowering=False, debug=False,
                   enable_asserts=False, num_devices=N_CORES)
    S_in = nc.dram_tensor("sin", (128, NC4, 12, 128), FP16,
                          kind="ExternalInput").ap()
    ones_in = nc.dram_tensor("ones", (128, 1), FP16, kind="ExternalInput").ap()
    ZB_o = nc.dram_tensor("zb", (128, 3 * N_UNITS), F32,
                          kind="ExternalOutput").ap()

    Exp = mybir.ActivationFunctionType.Exp

    with TileContext(nc) as tc:
        with (
            tc.tile_pool(name="src", bufs=1) as src_pool,
            tc.tile_pool(name="zps", bufs=1, space="PSUM") as zps_pool,
        ):
            ones = src_pool.tile([128, 1], FP16)
            nc.sync.dma_start(out=ones, in_=ones_in)
            Z = zps_pool.tile([128, 3 * N_UNITS], F32)

            for c in range(NC4):
                S = src_pool.tile([128, 12, 128], FP16, tag=f"S{c}")
                nc.sync.dma_start(out=S, in_=S_in[:, c])

                Epos = src_pool.tile([128, 8, 128], FP16, tag=f"Ep{c}")
                nc.scalar.activation(Epos, S[:, 0:8], Exp)
                Eneg = src_pool.tile([128, 10, 128], FP16, tag=f"En{c}")
                nc.scalar.activation(Eneg, S[:, 2:12], Exp, scale=-1.0)

                # base diffs for dap = xt - xs (on the Pool engine)
                rd = src_pool.tile([128, 1, 128], FP16, tag=f"rd{c}")
                nc.gpsimd.tensor_sub(rd, S[:, 0:1], S[:, 1:2])
                sd = src_pool.tile([128, 3, 128], FP16, tag=f"sd{c}")
                nc.gpsimd.tensor_sub(sd, S[:, 2:5], S[:, 5:8])
                dd1 = src_pool.tile([128, 3, 128], FP16, tag=f"dd{c}")
                nc.gpsimd.tensor_sub(dd1, rd.broadcast_to((128, 3, 128)), sd)

                def b3(ap, n):  # broadcast a [128,1,r] slice over n units
                    return ap.broadcast_to((128, n, 128))

                def jk(ap):  # [128,3|12,128] -> j,k expanded views
                    return ap

                # per-unit products; unit order: d1 j0..2, d2 k0..3, d3 (j,k)
                Pt = src_pool.tile([128, N_UNITS, 128], FP16, tag=f"Pt{c}")
                nc.vector.tensor_mul(Pt[:, 0:3], b3(Epos[:, 0:1], 3),
                                     Eneg[:, 0:3])
                nc.vector.tensor_mul(Pt[:, 3:7], b3(Epos[:, 0:1], 4),
                                     Eneg[:, 6:10])
                sht_e = Epos[:, 2:5].rearrange("p j (o r) -> p j o r", o=1) \
                    .broadcast_to((128, 3, 4, 128))
                simh_e = Eneg[:, 6:10].rearrange("p (o k) r -> p o k r", o=1) \
                    .broadcast_to((128, 3, 4, 128))
                Pt_d3 = Pt[:, 7:19].rearrange("p (j k) r -> p j k r", j=3)
                nc.vector.tensor_mul(Pt_d3, sht_e, simh_e)

                Ps = src_pool.tile([128, N_UNITS, 128], FP16, tag=f"Qs{c}")
                nc.vector.tensor_mul(Ps[:, 0:3], b3(Epos[:, 1:2], 3),
                                     Eneg[:, 3:6])
                nc.vector.tensor_mul(Ps[:, 3:7], b3(Epos[:, 1:2], 4),
                                     Eneg[:, 6:10])
                shs_e = Epos[:, 5:8].rearrange("p j (o r) -> p j o r", o=1) \
                    .broadcast_to((128, 3, 4, 128))
                Ps_d3 = Ps[:, 7:19].rearrange("p (j k) r -> p j k r", j=3)
                nc.vector.tensor_mul(Ps_d3, shs_e, simh_e)

                W = src_pool.tile([128, N_UNITS, 128], FP16, tag=f"W{c}")
                nc.vector.tensor_mul(W[:, 0:3], Pt[:, 0:3], dd1)
                nc.vector.tensor_mul(W[:, 3:7], Pt[:, 3:7], b3(rd, 4))
                sd_e = sd.rearrange("p j (o r) -> p j o r", o=1) \
                    .broadcast_to((128, 3, 4, 128))
                W_d3 = W[:, 7:19].rearrange("p (j k) r -> p j k r", j=3)
                nc.vector.tensor_mul(W_d3, Pt_d3, sd_e)

                # All 57 reductions over d on the PE: ones-vector matmuls,
                # accumulated across the NC4 partition chunks in PSUM.
                for t, T in enumerate((Pt, Ps, W)):
                    for u in range(N_UNITS):
                        col = t * N_UNITS + u
                        nc.tensor.matmul(
                            Z[:, col:col + 1],
                            lhsT=T[:, u, :],
                            rhs=ones,
                            start=(c == 0), stop=(c == NC4 - 1),
                            skip_group_check=True,
                        )
            zsb = src_pool.tile([128, 3 * N_UNITS], F32)
            nc.scalar.copy(zsb, Z)
            nc.sync.dma_start(out=ZB_o, in_=zsb)
    nc.compile()
    return nc


def _get(name):
    if name not in _CACHE:
        _CACHE[name] = _build_phase1() if name == "p1" else _build_phase2()
    return _CACHE[name]


def kernel(**inputs):
    tf = np.ascontiguousarray(np.asarray(inputs["teacher_feats"], dtype=np.float32))
    sf = np.ascontiguousarray(np.asarray(inputs["student_feats"], dtype=np.float32))
    in_dtype = np.asarray(inputs["ref_perm"]).dtype
    ref_perm = np.asarray(inputs["ref_perm"]).astype(np.int64)[:NUM_REF]
    shared_perm = np.asarray(inputs["shared_perm"]).astype(np.int64)[:NUM_SHARED]
    assert in_dtype == np.int32

    # ---- host gathers + normalization (tiny) ----
    ref_t = tf[:, 0, ref_perm, :]                       # [B, 256, 1024]
    ref_s = sf[:, 0, ref_perm, :]
    rn = np.sqrt(np.einsum("brd,brd->br", ref_t, ref_t))[..., None]
    refn = ref_t / np.maximum(rn, 1e-12)
    # scale by 64 so fp8e4m3 operates in its normal range
    refTs = [np.ascontiguousarray((refn[b].T * 64.0).astype(FP8_NP))
             for b in range(B)]

    # ---- phase 1: sharded cosine-sim + per-block top-8 ----
    in_maps1 = []
    for c in range(N_CORES):
        b, s = divmod(c, 4)
        x = tf[b, EXTRA_FRAMES[s]]                      # [4096, 1024]
        nrm = np.sqrt(np.einsum("ed,ed->e", x, x))
        xn = x / np.maximum(nrm, 1e-12)[:, None]
        extT = np.ascontiguousarray((xn.T * 64.0).astype(FP8_NP))
        in_maps1.append({"extT": extT, "refT": refTs[b]})

    res1 = bass_utils.run_bass_kernel_spmd(
        _get("p1"), in_maps1, core_ids=list(range(N_CORES)))
    LAST_PERF["p1"] = res1

    # ---- host cross-block/cross-shard top-k merge ----
    gidx = np.zeros((B, NUM_REF, TOPK), dtype=np.int64)
    for b in range(B):
        # vals [128, 2, NBLK, 8] per shard; row r = m*128 + p
        vals = np.concatenate(
            [res1.results[b * 4 + s]["vals"].astype(np.float32)
             .transpose(1, 0, 2, 3)
             .reshape(NUM_REF, NBLK * 8) for s in range(4)], axis=1)
        idxs = np.concatenate(
            [(res1.results[b * 4 + s]["idx"].astype(np.int64)
              + (np.arange(NBLK) * EB)[None, None, :, None]
              + s * ES).transpose(1, 0, 2, 3).reshape(NUM_REF, NBLK * 8)
             for s in range(4)], axis=1)
        order = np.argsort(-vals, axis=1, kind="stable")[:, :TOPK]
        gidx[b] = np.take_along_axis(idxs, order, axis=1)

    fr = np.asarray(EXTRA_FRAMES, dtype=np.int64)[gidx // P]
    pt = gidx % P
    sim_high = tf[np.arange(B)[:, None, None], fr, pt]  # [B, 256, 4, 1024]

    # ---- phase 2: distances ----
    sh_t = np.stack([tf[:, t, shared_perm, :] for t in SHARED_TEACHER], axis=1)
    sh_s = np.stack([sf[:, s, shared_perm, :] for s in SHARED_STUDENT], axis=1)

    def t_chunks(a):  # [128 rows, DH] -> [128 dpart, NC4, 128 rows]
        return a.T.reshape(NC4, 128, 128).transpose(1, 0, 2)

    ones_arr = np.ones((128, 1), dtype=np.float16)
    in_maps2 = []
    for c in range(N_CORES):
        b, h, dh = c >> 2, (c >> 1) & 1, c & 1
        rs = slice(h * 128, (h + 1) * 128)
        cs = slice(dh * DH, (dh + 1) * DH)
        # S slot order: rt, rs, sht0..2, shs0..2, simh0..3
        slots = ([ref_t[b, rs, cs], ref_s[b, rs, cs]]
                 + [sh_t[b, j, rs, cs] for j in range(3)]
                 + [sh_s[b, j, rs, cs] for j in range(3)]
                 + [sim_high[b, rs, k, cs] for k in range(4)])
        S = np.stack([t_chunks(a) for a in slots], axis=2)  # [128,NC4,12,128]
        in_maps2.append({"sin": np.ascontiguousarray(S.astype(np.float16)),
                         "ones": ones_arr})

    res2 = bass_utils.run_bass_kernel_spmd(
        _get("p2"), in_maps2, core_ids=list(range(N_CORES)))
    LAST_PERF["p2"] = res2

    # ---- host tail: kl + SmoothL1 + averaging ----
    s1 = s2 = s3 = 0.0
    for b in range(B):
        for h in range(2):
            z = (res2.results[b * 4 + h * 2 + 0]["zb"].astype(np.float64)
                 + res2.results[b * 4 + h * 2 + 1]["zb"].astype(np.float64))
            Zt = z[:, 0:N_UNITS]
            Zs = z[:, N_UNITS:2 * N_UNITS]
            num = z[:, 2 * N_UNITS:3 * N_UNITS]          # [128, 19]
            kl = num / Zt - np.log(Zt) + np.log(Zs)
            akl = np.abs(kl)
            hub = np.where(akl < BETA, 0.5 * kl * kl / BETA, akl - 0.5 * BETA)
            s1 += hub[:, 0:3].sum()
            s2 += hub[:, 3:7].sum()
            s3 += hub[:, 7:19].sum()

    loss = (s1 / (3 * B * NUM_REF)
            + s2 / (B * NUM_REF * TOPK)
            + s3 / (3 * B * NUM_REF * TOPK))
    return np.float32(loss)


# revision 9
# speedup vs baseline: 1.8793x; 1.0197x over previous
"""Trainium2 Bass kernel for nn_DA3CrossFrameCFDistanceLoss.

Strategy (8 NeuronCores):
  Phase 1 (data-parallel over batch x extra-frame shard):
    core c -> (b = c//4, shard s = c%4).  Host pre-normalizes both the ref
    rows and the extra-frame rows (folding the 1/||x|| scaling into the
    operands), scales by 64 and ships them as fp8e4m3, quartering HBM
    traffic.  Each core streams its frame transposed (extT [D, 4096] fp8),
    runs DoubleRow fp8 matmuls (256-deep contraction per instruction, 2x PE
    rate) against the stationary ref block with fp32 PSUM accumulation.
    The ACT engine copies each PSUM block to SBUF as bf16 and the DVE
    extracts per-512-column-block top-8 values + indices per ref row
    (max/max_index).  The host merges the 8 blocks x 4 shards top-8s per
    row into the global top-4 (selection is approximate only through the
    fp8/bf16 rounding of the similarity scores; the merge itself is exact)
    and gathers the selected rows at full fp32 precision.
  Phase 2 (data-parallel over (batch, row-half, feature-half)):
    core c -> (b, h, dh).  Layout is transposed on host: d sits on the
    partition axis (4 chunks of 128), rows on the free axis.  The 19 KL
    "units" need Zt = sum exp(xt), Zs = sum exp(xs), num = sum exp(xt)*
    (xt - xs).  Using exp(a-b) = exp(a)*exp(-b), the ACT engine computes
    just 18 base exponentials (2 wide instructions per chunk); the DVE
    forms the per-unit products Pt, Ps and W = Pt*(xt-xs) as wide
    stride-0-broadcast tensor ops; and the otherwise-idle PE performs all
    57 per-unit reductions over d as ones-vector matmuls accumulated
    across the 4 partition chunks in PSUM.  Everything is pipelined per
    d-chunk so DMA/ACT/DVE/Pool/PE overlap.  Host evaluates
    kl = num/Zt - log Zt + log Zs, SmoothL1 and the weighted averaging.
"""

import numpy as np
import ml_dtypes

import concourse.bass as bass
from concourse import bacc
import concourse.mybir as mybir
from concourse import bass_utils
from concourse.tile import TileContext

# ---- problem constants (hardcoded from the nn.Module defaults) ----
B, V, P, D = 2, 8, 4096, 1024
EXTRA_FRAMES = [1, 3, 5, 7]
SHARED_TEACHER = [2, 4, 6]
SHARED_STUDENT = [1, 2, 3]
NUM_REF = 256
NUM_SHARED = 256
TOPK = 4
TEMP = 1.0
BETA = 0.5
N_CORES = 8

ES = P          # extra rows per shard (one frame per shard)
EB = 512        # phase-1 e-block size (one PSUM bank of fp32)
NBLK = ES // EB
DH = D // 2     # phase-2 feature half
NC4 = DH // 128  # phase-2 d chunks per core
N_UNITS = 19    # 3 d1 + 4 d2 + 12 d3

F32 = mybir.dt.float32
BF16 = mybir.dt.bfloat16
FP16 = mybir.dt.float16
FP8 = mybir.dt.float8e4
U32 = mybir.dt.uint32

BF16_NP = ml_dtypes.bfloat16
FP8_NP = mybir.dt.np(FP8)

_CACHE = {}

# Results of the most recent launches (exec_time_ns etc), for test harnesses.
LAST_PERF = {}


def _build_phase1():
    nc = bacc.Bacc("TRN2", target_bir_lowering=False, debug=False,
                   enable_asserts=False, num_devices=N_CORES)
    extT = nc.dram_tensor("extT", (D, ES), FP8, kind="ExternalInput").ap()
    refT = nc.dram_tensor("refT", (D, NUM_REF), FP8, kind="ExternalInput").ap()
    vals_o = nc.dram_tensor("vals", (128, 2, NBLK, 8), BF16,
                            kind="ExternalOutput").ap()
    idx_o = nc.dram_tensor("idx", (128, 2, NBLK, 8), U32,
                           kind="ExternalOutput").ap()

    extT_r = extT.rearrange("(k p) e -> p k e", p=128)
    refT_r = refT.rearrange("(k p) r -> p k r", p=128)
    DR = mybir.MatmulPerfMode.DoubleRow

    with TileContext(nc) as tc:
        with (
            tc.tile_pool(name="const", bufs=1) as const_pool,
            tc.tile_pool(name="xin", bufs=3) as xin_pool,
            tc.tile_pool(name="sb", bufs=4) as sb_pool,
            tc.tile_pool(name="ps", bufs=4, space="PSUM") as ps_pool,
            tc.tile_pool(name="dum", bufs=1, space="PSUM") as dum_pool,
            tc.tile_pool(name="small", bufs=1) as small_pool,
        ):
            refT_sb = const_pool.tile([128, 8, NUM_REF], FP8)
            nc.sync.dma_start(out=refT_sb, in_=refT_r)
            fv = small_pool.tile([128, 2, NBLK, 8], BF16)
            fidx = small_pool.tile([128, 2, NBLK, 8], U32)
            dum = dum_pool.tile([128, 8], F32)

            for eb in range(NBLK):
                esl = slice(eb * EB, (eb + 1) * EB)
                xt = xin_pool.tile([128, 8, EB], FP8, tag="xt")
                nc.sync.dma_start(out=xt, in_=extT_r[:, :, esl])
                # A PE matmul may carry at most one semaphore wait (walrus
                # S3_LW limit).  Consume the xt-DMA dependency with a tiny
                # throwaway matmul so the real matmuls only ever wait on one
                # source (the PSUM tile WAR release).
                nc.tensor.matmul(dum, lhsT=xt[:, 0, 0:128], rhs=xt[:, 0, 0:8],
                                 start=True, stop=True, skip_group_check=True)
                for m in range(2):
                    ps = ps_pool.tile([128, EB], F32, tag="ps", name="ps")
                    for k2 in range(4):
                        nc.tensor.matmul(
                            ps,
                            lhsT=refT_sb[:, 2 * k2:2 * k2 + 2,
                                         m * 128:(m + 1) * 128],
                            rhs=xt[:, 2 * k2:2 * k2 + 2, :],
                            start=(k2 == 0), stop=(k2 == 3),
                            perf_mode=DR,
                        )
                    sim = sb_pool.tile([128, EB], BF16, tag="sim", name="sim")
                    nc.scalar.copy(sim, ps)
                    nc.vector.max(out=fv[:, m, eb, :], in_=sim)
                    nc.vector.max_index(out=fidx[:, m, eb, :],
                                        in_max=fv[:, m, eb, :], in_values=sim)
            nc.sync.dma_start(out=vals_o, in_=fv)
            nc.sync.dma_start(out=idx_o, in_=fidx)
    nc.compile()
    return nc


def _build_phase2():
    # Transposed layout: partition axis = d (4 chunks of 128), free = rows.
    # Input S slot order: rt=0, rs=1, sht=2..4, shs=5..7, simh=8..11.
    # Epos slots (exp of S[0:8]):  rt=0, rs=1, sht=2..4, shs=5..7
    # Eneg slots (exp of -S[2:12]): sht=0..2, shs=3..5, simh=6..9
    nc = bacc.Bacc("TRN2", target_bir_lowering=False, debug=False,
                   enable_asserts=False, num_devices=N_CORES)
    S_in = nc.dram_tensor("sin", (128, NC4, 12, 128), FP16,
                          kind="ExternalInput").ap()
    ones_in = nc.dram_tensor("ones", (128, 1), FP16, kind="ExternalInput").ap()
    ZB_o = nc.dram_tensor("zb", (128, NC4, 3 * N_UNITS), F32,
                          kind="ExternalOutput").ap()

    Exp = mybir.ActivationFunctionType.Exp

    with TileContext(nc) as tc:
        with (
            tc.tile_pool(name="src", bufs=1) as src_pool,
            tc.tile_pool(name="zps", bufs=1, space="PSUM") as zps_pool,
        ):
            ones = src_pool.tile([128, 1], FP16)
            nc.sync.dma_start(out=ones, in_=ones_in)
            # One column per (chunk, tensor, unit): every matmul is its own
            # start+stop group, so the PSUM bank-granular pending-zero of
            # start_tensor_calc never clobbers previously accumulated
            # columns.  Host sums the NC4 chunk columns.
            Z = zps_pool.tile([128, NC4, 3 * N_UNITS], F32)

            for c in range(NC4):
                S = src_pool.tile([128, 12, 128], FP16, tag=f"S{c}")
                nc.sync.dma_start(out=S, in_=S_in[:, c])

                Epos = src_pool.tile([128, 8, 128], FP16, tag=f"Ep{c}")
                nc.scalar.activation(Epos, S[:, 0:8], Exp)
                Eneg = src_pool.tile([128, 10, 128], FP16, tag=f"En{c}")
                nc.scalar.activation(Eneg, S[:, 2:12], Exp, scale=-1.0)

                # base diffs for dap = xt - xs (on the Pool engine)
                rd = src_pool.tile([128, 1, 128], FP16, tag=f"rd{c}")
                nc.gpsimd.tensor_sub(rd, S[:, 0:1], S[:, 1:2])
                sd = src_pool.tile([128, 3, 128], FP16, tag=f"sd{c}")
                nc.gpsimd.tensor_sub(sd, S[:, 2:5], S[:, 5:8])
                dd1 = src_pool.tile([128, 3, 128], FP16, tag=f"dd{c}")
                nc.gpsimd.tensor_sub(dd1, rd.broadcast_to((128, 3, 128)), sd)

                def b3(ap, n):  # broadcast a [128,1,r] slice over n units
                    return ap.broadcast_to((128, n, 128))

                def jk(ap):  # [128,3|12,128] -> j,k expanded views
                    return ap

                # per-unit products; unit order: d1 j0..2, d2 k0..3, d3 (j,k)
                Pt = src_pool.tile([128, N_UNITS, 128], FP16, tag=f"Pt{c}")
                nc.vector.tensor_mul(Pt[:, 0:3], b3(Epos[:, 0:1], 3),
                                     Eneg[:, 0:3])
                nc.vector.tensor_mul(Pt[:, 3:7], b3(Epos[:, 0:1], 4),
                                     Eneg[:, 6:10])
                sht_e = Epos[:, 2:5].rearrange("p j (o r) -> p j o r", o=1) \
                    .broadcast_to((128, 3, 4, 128))
                simh_e = Eneg[:, 6:10].rearrange("p (o k) r -> p o k r", o=1) \
                    .broadcast_to((128, 3, 4, 128))
                Pt_d3 = Pt[:, 7:19].rearrange("p (j k) r -> p j k r", j=3)
                nc.vector.tensor_mul(Pt_d3, sht_e, simh_e)

                Ps = src_pool.tile([128, N_UNITS, 128], FP16, tag=f"Qs{c}")
                nc.vector.tensor_mul(Ps[:, 0:3], b3(Epos[:, 1:2], 3),
                                     Eneg[:, 3:6])
                nc.vector.tensor_mul(Ps[:, 3:7], b3(Epos[:, 1:2], 4),
                                     Eneg[:, 6:10])
                shs_e = Epos[:, 5:8].rearrange("p j (o r) -> p j o r", o=1) \
                    .broadcast_to((128, 3, 4, 128))
                Ps_d3 = Ps[:, 7:19].rearrange("p (j k) r -> p j k r", j=3)
                nc.vector.tensor_mul(Ps_d3, shs_e, simh_e)

                W = src_pool.tile([128, N_UNITS, 128], FP16, tag=f"W{c}")
                nc.vector.tensor_mul(W[:, 0:3], Pt[:, 0:3], dd1)
                nc.vector.tensor_mul(W[:, 3:7], Pt[:, 3:7], b3(rd, 4))
                sd_e = sd.rearrange("p j (o r) -> p j o r", o=1) \
                    .broadcast_to((128, 3, 4, 128))
                W_d3 = W[:, 7:19].rearrange("p (j k) r -> p j k r", j=3)
                nc.vector.tensor_mul(W_d3, Pt_d3, sd_e)

                # All 57 reductions over d on the PE: ones-vector matmuls,
                # accumulated across the NC4 partition chunks in PSUM.
                for t, T in enumerate((Pt, Ps, W)):
                    for u in range(N_UNITS):
                        col = t * N_UNITS + u
                        nc.tensor.matmul(
                            Z[:, c, col:col + 1],
                            lhsT=T[:, u, :],
                            rhs=ones,
                            start=True, stop=True,
                            skip_group_check=True,
                        )
            zsb = src_pool.tile([128, NC4, 3 * N_UNITS], F32)
            nc.scalar.copy(zsb, Z)
            nc.sync.dma_start(out=ZB_o, in_=zsb)
    nc.compile()
    return nc


def _get(name):
    if name not in _CACHE:
        _CACHE[name] = _build_phase1() if name == "p1" else _build_phase2()
    return _CACHE[name]


def kernel(**inputs):
    tf = np.ascontiguousarray(np.asarray(inputs["teacher_feats"], dtype=np.float32))
    sf = np.ascontiguousarray(np.asarray(inputs["student_feats"], dtype=np.float32))
    in_dtype = np.asarray(inputs["ref_perm"]).dtype
    ref_perm = np.asarray(inputs["ref_perm"]).astype(np.int64)[:NUM_REF]
    shared_perm = np.asarray(inputs["shared_perm"]).astype(np.int64)[:NUM_SHARED]
    assert in_dtype == np.int32

    # ---- host gathers + normalization (tiny) ----
    ref_t = tf[:, 0, ref_perm, :]                       # [B, 256, 1024]
    ref_s = sf[:, 0, ref_perm, :]
    rn = np.sqrt(np.einsum("brd,brd->br", ref_t, ref_t))[..., None]
    refn = ref_t / np.maximum(rn, 1e-12)
    # scale by 64 so fp8e4m3 operates in its normal range
    refTs = [np.ascontiguousarray((refn[b].T * 64.0).astype(FP8_NP))
             for b in range(B)]

    # ---- phase 1: sharded cosine-sim + per-block top-8 ----
    in_maps1 = []
    for c in range(N_CORES):
        b, s = divmod(c, 4)
        x = tf[b, EXTRA_FRAMES[s]]                      # [4096, 1024]
        nrm = np.sqrt(np.einsum("ed,ed->e", x, x))
        xn = x / np.maximum(nrm, 1e-12)[:, None]
        extT = np.ascontiguousarray((xn.T * 64.0).astype(FP8_NP))
        in_maps1.append({"extT": extT, "refT": refTs[b]})

    res1 = bass_utils.run_bass_kernel_spmd(
        _get("p1"), in_maps1, core_ids=list(range(N_CORES)))
    LAST_PERF["p1"] = res1

    # ---- host cross-block/cross-shard top-k merge ----
    gidx = np.zeros((B, NUM_REF, TOPK), dtype=np.int64)
    for b in range(B):
        # vals [128, 2, NBLK, 8] per shard; row r = m*128 + p
        vals = np.concatenate(
            [res1.results[b * 4 + s]["vals"].astype(np.float32)
             .transpose(1, 0, 2, 3)
             .reshape(NUM_REF, NBLK * 8) for s in range(4)], axis=1)
        idxs = np.concatenate(
            [(res1.results[b * 4 + s]["idx"].astype(np.int64)
              + (np.arange(NBLK) * EB)[None, None, :, None]
              + s * ES).transpose(1, 0, 2, 3).reshape(NUM_REF, NBLK * 8)
             for s in range(4)], axis=1)
        order = np.argsort(-vals, axis=1, kind="stable")[:, :TOPK]
        gidx[b] = np.take_along_axis(idxs, order, axis=1)

    fr = np.asarray(EXTRA_FRAMES, dtype=np.int64)[gidx // P]
    pt = gidx % P
    sim_high = tf[np.arange(B)[:, None, None], fr, pt]  # [B, 256, 4, 1024]

    # ---- phase 2: distances ----
    sh_t = np.stack([tf[:, t, shared_perm, :] for t in SHARED_TEACHER], axis=1)
    sh_s = np.stack([sf[:, s, shared_perm, :] for s in SHARED_STUDENT], axis=1)

    def t_chunks(a):  # [128 rows, DH] -> [128 dpart, NC4, 128 rows]
        return a.T.reshape(NC4, 128, 128).transpose(1, 0, 2)

    ones_arr = np.ones((128, 1), dtype=np.float16)
    in_maps2 = []
    for c in range(N_CORES):
        b, h, dh = c >> 2, (c >> 1) & 1, c & 1
        rs = slice(h * 128, (h + 1) * 128)
        cs = slice(dh * DH, (dh + 1) * DH)
        # S slot order: rt, rs, sht0..2, shs0..2, simh0..3
        slots = ([ref_t[b, rs, cs], ref_s[b, rs, cs]]
                 + [sh_t[b, j, rs, cs] for j in range(3)]
                 + [sh_s[b, j, rs, cs] for j in range(3)]
                 + [sim_high[b, rs, k, cs] for k in range(4)])
        S = np.stack([t_chunks(a) for a in slots], axis=2)  # [128,NC4,12,128]
        in_maps2.append({"sin": np.ascontiguousarray(S.astype(np.float16)),
                         "ones": ones_arr})

    res2 = bass_utils.run_bass_kernel_spmd(
        _get("p2"), in_maps2, core_ids=list(range(N_CORES)))
    LAST_PERF["p2"] = res2

    # ---- host tail: kl + SmoothL1 + averaging ----
    s1 = s2 = s3 = 0.0
    for b in range(B):
        for h in range(2):
            z = (res2.results[b * 4 + h * 2 + 0]["zb"].astype(np.float64)
                 + res2.results[b * 4 + h * 2 + 1]["zb"].astype(np.float64)
                 ).sum(axis=1)
            Zt = z[:, 0:N_UNITS]
            Zs = z[:, N_UNITS:2 * N_UNITS]
            num = z[:, 2 * N_UNITS:3 * N_UNITS]          # [128, 19]
            kl = num / Zt - np.log(Zt) + np.log(Zs)
            akl = np.abs(kl)
            hub = np.where(akl < BETA, 0.5 * kl * kl / BETA, akl - 0.5 * BETA)
            s1 += hub[:, 0:3].sum()
            s2 += hub[:, 3:7].sum()
            s3 += hub[:, 7:19].sum()

    loss = (s1 / (3 * B * NUM_REF)
            + s2 / (B * NUM_REF * TOPK)
            + s3 / (3 * B * NUM_REF * TOPK))
    return np.float32(loss)
